# revision 1
# baseline (speedup 1.0000x reference)
"""Trainium2 Bass kernel for nn_Discriminator_55800215109843.

Model: 4x (Conv2d k3 s2 p1 + LeakyReLU(0.2) [+ BatchNorm eval]) on
[128,3,128,128] -> [128,128,8,8], then a 50-step LIF neuron scan
(beta=0.95, thr=1, subtract reset) whose spike record feeds a linear
layer [409600 -> 1] + sigmoid.

Strategy (8 NeuronCores, pure data parallelism over batch, 16 imgs/core):
  * Convs as tap-accumulation matmuls: channels (x images, block-diag
    weights) on the contraction dim, strided access-pattern views of
    zero-padded SBUF planes for the 9 taps; PSUM accumulation.
  * LeakyReLU(0.2) via lrelu(x) = x - 0.8*relu(-x): ACT Relu pass +
    one fused DVE scalar_tensor_tensor pass. BN (eval) is folded into
    conv weights/biases on the host.
  * LIF scan in layout [c=128 partitions, (b=16,hw=64) free]: 2 fused
    DVE STT passes per step (u = 0.95*m + (c-0.5); m = u - 0.5*r), the
    spike sign r = sign(m-1) on the otherwise-idle ACT engine, and the
    linear layer folded INTO the scan as 50 accumulating PE matmuls
    (float32r, full rate) against the +-1 r tiles; the hw-diagonal of
    the [64,1024] PSUM result plus the sum-of-wl constant recover the
    0/1-spike dot product on the host.
  * Device matmuls avoid rapidly alternating tile_position row bases
    (0 <-> 64) -- that pattern hard-crashes the device; each layer
    issues all base-0 groups, then all base-64 groups.
"""

import sys

sys.path.insert(0, "/opt/trn_rl_repo")

import numpy as np

import concourse.bass as bass
import concourse.mybir as mybir
import concourse.tile as tile
from concourse import bacc
from concourse.bass_utils import run_bass_kernel_spmd

F32 = mybir.dt.float32
F32R = mybir.dt.float32r
BF16 = mybir.dt.bfloat16
OP = mybir.AluOpType
AF = mybir.ActivationFunctionType

N_CORES = 8
B_FULL = 128
B_LOC = 16          # images per core
T = 50              # LIF steps
BETA = 0.95
S = 128             # input spatial

# layer configs: (C_in, C_out, H_in, n_img per matmul group)
# K = n_img*C_in (<=128), M = n_img*C_out (<=128)
L1 = dict(ci=3, co=16, hin=128, ni=8)
L2 = dict(ci=16, co=32, hin=64, ni=4)
L3 = dict(ci=32, co=64, hin=16 * 2, ni=2)
L4 = dict(ci=64, co=128, hin=16, ni=1)


def _np(x):
    return np.ascontiguousarray(np.asarray(x, dtype=np.float32))


def _fold_bn(g, bb, rm, rv, eps=0.8):
    scale = g / np.sqrt(rv + eps)
    shift = bb - rm * scale
    return scale.astype(np.float32), shift.astype(np.float32)


def _block_diag_taps(w, n_img, col_scale=None):
    """w: [C_out, C_in, 3, 3] -> taps [9, 128, 128] block-diag over n_img
    images, duplicated at row offset 64 for tile_position row pairing.

    rows: 64*h + (i_loc*C_in + c)   (h in {0,1} duplicate halves)
    cols: i_loc*C_out + c_out
    """
    co, ci = w.shape[0], w.shape[1]
    k = n_img * ci
    m = n_img * co
    assert k <= 64 or n_img == 1, (k, n_img)
    assert m <= 128
    taps = np.zeros((9, 128, 128), np.float32)
    for tp in range(9):
        dy, dx = tp // 3, tp % 3
        blk = w[:, :, dy, dx].T.astype(np.float32)  # [ci, co]
        if col_scale is not None:
            blk = blk * col_scale[None, :]
        for i in range(n_img):
            taps[tp, i * ci : (i + 1) * ci, i * co : (i + 1) * co] = blk
        if k <= 64:
            taps[tp, 64 : 64 + k, :] = taps[tp, :k, :]
    return taps


def _l1_dyrep_taps(w):
    """w1 [16, 3, 3, 3] -> dx-taps [3, 128, 128], rows (dy*24 + i*3 + c),
    cols (i*16 + c_out), block-diag over 8 images."""
    taps = np.zeros((3, 128, 128), np.float32)
    for dx in range(3):
        for dy in range(3):
            blk = w[:, :, dy, dx].T.astype(np.float32)  # [3, 16]
            for i in range(8):
                taps[dx, dy * 24 + i * 3 : dy * 24 + i * 3 + 3,
                     i * 16 : (i + 1) * 16] = blk
    return taps


def _bias_vec(b, n_img):
    v = np.zeros((128, 1), np.float32)
    co = b.shape[0]
    for i in range(n_img):
        v[i * co : (i + 1) * co, 0] = b
    return v


def build_nc(sigma_engine="sign", debug_dump=False, stages="full",
             f32r_layers=frozenset()):
    LDT = {li: (F32R if li in f32r_layers else F32) for li in (1, 2, 3, 4)}
    nc = bacc.Bacc("TRN2", target_bir_lowering=False, debug=False)

    # ---------------- DRAM I/O ----------------
    img_d = nc.dram_tensor("img", [B_LOC, 3, S, S], LDT[1], kind="ExternalInput")
    w_d = {}
    for li in (1, 2, 3, 4):
        w_d[li] = nc.dram_tensor(f"w{li}t", [9, 128, 128], LDT[li], kind="ExternalInput")
    bp_d = nc.dram_tensor("biasp", [4, 128], F32, kind="ExternalInput")  # for DVE pass
    bn_d = nc.dram_tensor("biasn", [4, 128], F32, kind="ExternalInput")  # -0.8*b for ACT
    wl_d = nc.dram_tensor("wlt", [128, T * 64], F32R, kind="ExternalInput")
    out_d = nc.dram_tensor("D", [64, 1024], F32, kind="ExternalOutput")
    dbg_d = {}
    if debug_dump:
        dbg_d["x2_0"] = nc.dram_tensor("dbg_x2_0", [128, 66 * 66], F32, kind="ExternalOutput")
        dbg_d["x3_0"] = nc.dram_tensor("dbg_x3_0", [128, 34 * 34], F32, kind="ExternalOutput")
        dbg_d["x4_0"] = nc.dram_tensor("dbg_x4_0", [128, 18 * 18], F32, kind="ExternalOutput")
        dbg_d["ctile"] = nc.dram_tensor("dbg_ctile", [128, 1024], F32, kind="ExternalOutput")

    with tile.TileContext(nc) as tc:
        with (
            tc.tile_pool(name="const", bufs=1) as constp,
            tc.tile_pool(name="acts", bufs=1) as acts,
            tc.tile_pool(name="tmps", bufs=3) as tmps,
            tc.tile_pool(name="psum", bufs=3, space="PSUM") as psp,
            tc.tile_pool(name="psl4", bufs=1, space="PSUM") as psl4,
        ):
            # ---------------- load constants ----------------
            wsb = {}
            for li in (1, 2, 3, 4):
                wsb[li] = constp.tile([128, 9, 128], LDT[li], name=f"w{li}sb", tag=f"w{li}sb")
                nc.sync.dma_start(wsb[li][:], w_d[li].ap().transpose([1, 0, 2]))
            biasp = constp.tile([128, 4], F32, name="biasp", tag="biasp")
            nc.sync.dma_start(biasp[:], bp_d.ap().transpose([1, 0]))
            biasn = constp.tile([128, 4], F32, name="biasn", tag="biasn")
            nc.sync.dma_start(biasn[:], bn_d.ap().transpose([1, 0]))
            wl = constp.tile([128, T * 64], F32R, name="wl", tag="wl")
            nc.sync.dma_start(wl[:], wl_d.ap())

            # ---------------- activation planes ----------------
            # x1: [88, 130*130] two 8-img groups at partition 0 / 64
            # x2: 2 tiles [128=(i8,c16), 66*66]
            # x3: 4 tiles [128=(i4,c32), 34*34]
            # x4: 8 tiles [128=(i2,c64), 18*18]
            x1 = acts.tile([88, 130 * 130], LDT[1], name="x1", tag="x1")
            x2 = [acts.tile([128, 66 * 66], LDT[2], name=f"x2_{i}", tag=f"x2_{i}") for i in range(2)]
            x3 = [acts.tile([128, 34 * 34], LDT[3], name=f"x3_{i}", tag=f"x3_{i}") for i in range(4)]
            x4 = [acts.tile([128, 18 * 18], LDT[4], name=f"x4_{i}", tag=f"x4_{i}") for i in range(8)]
            ctile = acts.tile([128, 1024], F32, name="ctile", tag="ctile")

            def zero_borders(t, npart, hp):
                v = t[:].bitcast(F32).rearrange("p (h w) -> p h w", w=hp)[0:npart]
                nc.vector.memset(v[:, 0, :], 0.0)
                nc.vector.memset(v[:, hp - 1, :], 0.0)
                nc.vector.memset(v[:, 1 : hp - 1, 0], 0.0)
                nc.vector.memset(v[:, 1 : hp - 1, hp - 1], 0.0)

            # borders of padded planes
            for g in range(2):
                v = x1[:].bitcast(F32).rearrange("p (h w) -> p h w", w=130)
                base = 64 * g
                nc.vector.memset(v[base : base + 24, 0, :], 0.0)
                nc.vector.memset(v[base : base + 24, 129, :], 0.0)
                nc.vector.memset(v[base : base + 24, 1:129, 0], 0.0)
                nc.vector.memset(v[base : base + 24, 1:129, 129], 0.0)
            for t in x2:
                zero_borders(t, 128, 66)
            for t in x3:
                zero_borders(t, 128, 34)
            for t in x4:
                zero_borders(t, 128, 18)

            # ---------------- input DMA ----------------
            for g in range(2):
                srcap = bass.AP(
                    tensor=img_d,
                    offset=g * 8 * 3 * S * S,
                    ap=[[S * S, 24], [S, S], [1, S]],
                )
                dst = x1[:].rearrange("p (h w) -> p h w", w=130)[
                    64 * g : 64 * g + 24, 1:129, 1:129
                ]
                nc.sync.dma_start(dst, srcap)

            # ---------------- conv layers ----------------
            def conv_layer(wtile, rhs_of, psum_sets, emit_out):
                """Generic tap-accumulation conv.

                rhs_of(gi, tap, q) -> (rhs AP, tile_position)
                psum_sets: list of (gi, q) output chunk ids
                emit_out(gi, q, ps_flat): epilogue on filled psum slice
                """
                for gi, q in psum_sets:
                    ps = psp.tile([128, 512], F32, name="convps", tag="convps")
                    n = None
                    for tp in range(9):
                        rhs, tpos = rhs_of(gi, tp, q)
                        kk = rhs.partition_size()
                        n = rhs.free_size()
                        lhsT = wtile[tpos[0] : tpos[0] + kk, tp, :]
                        nc.tensor.matmul(
                            ps[:, 0:n],
                            lhsT,
                            rhs,
                            start=(tp == 0),
                            stop=(tp == 8),
                            tile_position=tpos,
                        )
                    emit_out(gi, q, ps[:, 0:n])

            def epilogue(ps, out_ap, bias_idx):
                """x = ps + bias; out = x + 0.8*relu(-x) == lrelu(x)."""
                n_free = ps.free_size()
                r = tmps.tile([128, 512], F32, name="relu_tmp", tag="relu_tmp")
                rr = r[:, 0:n_free]
                nc.scalar.activation(
                    rr,
                    ps,
                    AF.Relu,
                    bias=biasn[:, bias_idx : bias_idx + 1],
                    scale=-0.8,
                )
                nc.vector.scalar_tensor_tensor(
                    out_ap,
                    ps,
                    biasp[:, bias_idx : bias_idx + 1],
                    rr,
                    OP.add,
                    OP.add,
                )

            # ---- L1: groups g in {0,1} (8 imgs), 8 col chunks of 512 ----
            x1v = x1[:].rearrange("p (h w) -> p h w", w=130)

            def l1_rhs(g, tp, q):
                dy, dx = tp // 3, tp % 3
                base = 64 * g
                rows = 16 * q + dy
                rhs = x1v[base : base + 24, rows : rows + 16 : 2, dx : dx + 128 : 2]
                return rhs, (base, 0)

            def l1_out(g, q, ps):
                # psum [128=(i8,co16), (yy8, x64)] -> x2[g] interior rows 8q..8q+8
                dst = x2[g][:].rearrange("p (h w) -> p h w", w=66)[
                    :, 8 * q + 1 : 8 * q + 9, 1:65
                ]
                epilogue(ps, dst, 0)

            conv_layer(
                wsb[1], l1_rhs, [(g, q) for g in range(2) for q in range(8)], l1_out
            )

            if stages == "l1":
                zz = acts.tile([64, 1024], F32, name="zz", tag="zz")
                nc.vector.tensor_copy(zz[:], x2[1][64:128, 0:1024])
                nc.sync.dma_start(out_d.ap(), zz[:])

            lvl = {"l1": 1, "l2": 2, "l3": 3, "conv": 4, "full": 5}[stages]

            # ---- L2: groups g2 in {0..3} (4 imgs), 2 col chunks of 512 ----
            def l2_rhs(g2, tp, q):
                dy, dx = tp // 3, tp % 3
                v = x2[g2 // 2][:].rearrange("p (h w) -> p h w", w=66)
                base = 64 * (g2 % 2)
                rows = 32 * q + dy
                rhs = v[base : base + 64, rows : rows + 32 : 2, dx : dx + 64 : 2]
                return rhs, (base, 0)

            def l2_out(g2, q, ps):
                # psum [(i4,co32), (yy16, x32)] -> x3[g2] rows 16q..16q+16
                dst = x3[g2][:].rearrange("p (h w) -> p h w", w=34)[
                    :, 16 * q + 1 : 16 * q + 17, 1:33
                ]
                epilogue(ps, dst, 1)

            if lvl >= 2:
                conv_layer(
                    wsb[2], l2_rhs,
                    [(g, q) for g in (0, 2, 1, 3) for q in range(2)], l2_out
                )

            # ---- L3: groups g3 in {0..7} (2 imgs), one 256-col chunk ----
            def l3_rhs(g3, tp, q):
                dy, dx = tp // 3, tp % 3
                v = x3[g3 // 2][:].rearrange("p (h w) -> p h w", w=34)
                base = 64 * (g3 % 2)
                rhs = v[base : base + 64, dy : dy + 32 : 2, dx : dx + 32 : 2]
                return rhs, (base, 0)

            def l3_out(g3, q, ps):
                dst = x4[g3][:].rearrange("p (h w) -> p h w", w=18)[
                    :, 1:17, 1:17
                ]
                epilogue(ps, dst, 2)

            if stages == "l2":
                zz2 = acts.tile([64, 1024], F32, name="zz2", tag="zz2")
                nc.vector.tensor_copy(zz2[:], x3[0][0:64, 0:1024])
                nc.sync.dma_start(out_d.ap(), zz2[:])

            if lvl >= 3:
                conv_layer(
                    wsb[3], l3_rhs, [(g, 0) for g in (0, 2, 4, 6, 1, 3, 5, 7)], l3_out
                )

            if stages == "l3":
                zz3 = acts.tile([64, 1024], F32, name="zz3", tag="zz3")
                nc.vector.memset(zz3[:], 0.0)
                nc.vector.tensor_copy(zz3[:, 0:324], x4[0][0:64, 0:324])
                nc.sync.dma_start(out_d.ap(), zz3[:])

            # ---- L4: 16 imgs, 64 cols each, 2 long-lived psum banks ----
            if lvl >= 4:
                ps4 = [psl4.tile([128, 512], F32, name=f"ps4_{i}", tag=f"ps4_{i}") for i in range(2)]
                for ii in [0, 2, 4, 6, 8, 10, 12, 14, 1, 3, 5, 7, 9, 11, 13, 15]:
                    v = x4[ii // 2][:].rearrange("p (h w) -> p h w", w=18)
                    base = 64 * (ii % 2)
                    for tp in range(9):
                        dy, dx = tp // 3, tp % 3
                        rhs = v[base : base + 64, dy : dy + 16 : 2, dx : dx + 16 : 2]
                        lhsT = wsb[4][base : base + 64, tp, :]
                        nc.tensor.matmul(
                            ps4[ii // 8][:, 64 * (ii % 8) : 64 * (ii % 8) + 64],
                            lhsT,
                            rhs,
                            start=(tp == 0),
                            stop=(tp == 8),
                            tile_position=(base, 0),
                            skip_group_check=True,
                        )
                # epilogue -> ctile [c128, (b16, hw64)]
                for pb in range(2):
                    epilogue(ps4[pb][:], ctile[:, 512 * pb : 512 * pb + 512], 3)

            if debug_dump:
                nc.sync.dma_start(dbg_d["x2_0"].ap(), x2[0][:])
                nc.sync.dma_start(dbg_d["x3_0"].ap(), x3[0][:])
                nc.sync.dma_start(dbg_d["x4_0"].ap(), x4[0][:])
                nc.sync.dma_start(dbg_d["ctile"].ap(), ctile[:])

            if stages == "conv":
                zz = acts.tile([64, 1024], F32, name="zz", tag="zz")
                nc.vector.tensor_copy(zz[:], ctile[0:64, :])
                nc.sync.dma_start(out_d.ap(), zz[:])

            # ---------------- LIF scan + folded linear ----------------
            if lvl >= 5:
              with (
                tc.tile_pool(name="scan", bufs=1) as scp,
                tc.tile_pool(name="psd", bufs=1, space="PSUM") as psd,
              ):
                m = scp.tile([128, 1024], F32, name="m", tag="m")
                u = scp.tile([128, 1024], F32, name="u", tag="u")
                cp = scp.tile([128, 1024], F32, name="cp", tag="cp")
                # r = sign(m - 1) in {-1, +1}: sigma = (r + 1) / 2
                sig = [scp.tile([128, 1024], F32R, name=f"sig{i}", tag=f"sig{i}") for i in range(2)]
                d0 = psd.tile([64, 512], F32, name="d0", tag="d0")
                d1 = psd.tile([64, 512], F32, name="d1", tag="d1")

                neg1 = scp.tile([128, 1], F32, name="neg1", tag="neg1")
                nc.vector.memset(neg1[:], -1.0)
                nc.vector.memset(m[:], 0.0)
                nc.vector.memset(sig[1][:].bitcast(F32), -1.0 if sigma_engine == "sign" else 0.0)
                # cp = c - 0.5 (folds the (r+1)/2 offset into the input)
                nc.vector.tensor_scalar_sub(cp[:], ctile[:], 0.5)

                use_sign = sigma_engine == "sign"
                # Column split: DVE owns [0:G), GPSIMD [G:1024) of both fused
                # per-step passes (elementwise in the free dim, so the split
                # is dependency-free). GPSIMD 2-input ops run ~half DVE rate,
                # so it gets ~1/4 of the columns.
                G = 1024  # gpsimd lacks the fused STT opcode on HW
                cin = cp if use_sign else ctile
                rk = -0.5 if use_sign else -1.0
                for t in range(T):
                    rprev = sig[(t + 1) % 2]
                    rcur = sig[t % 2]
                    # u = 0.95*m + (c - 0.5|0)
                    nc.vector.scalar_tensor_tensor(
                        u[:, 0:G], m[:, 0:G], BETA, cin[:, 0:G], OP.mult, OP.add
                    )
                    if G < 1024:
                        nc.gpsimd.scalar_tensor_tensor(
                            u[:, G:1024], m[:, G:1024], BETA, cin[:, G:1024],
                            OP.mult, OP.add
                        )
                    # m = (-0.5*r_prev | -1*sigma_prev) + u
                    nc.vector.scalar_tensor_tensor(
                        m[:, 0:G], rprev[:, 0:G], rk, u[:, 0:G], OP.mult, OP.add
                    )
                    if G < 1024:
                        nc.gpsimd.scalar_tensor_tensor(
                            m[:, G:1024], rprev[:, G:1024], rk, u[:, G:1024],
                            OP.mult, OP.add
                        )
                    if use_sign:
                        # r_t = sign(m - 1)  (ACT engine, hidden under DVE)
                        nc.scalar.activation(rcur[:], m[:], AF.Sign, bias=neg1[:])
                    else:
                        nc.vector.tensor_scalar(rcur[:], m[:], 1.0, None, OP.is_gt)
                    # D += sum_c wl[c,t,hw_w] * r[c,(b,hw_r)]
                    nc.tensor.matmul(
                        d0[:],
                        wl[:, 64 * t : 64 * t + 64],
                        rcur[:, 0:512],
                        start=(t == 0),
                        stop=(t == T - 1),
                    )
                    nc.tensor.matmul(
                        d1[:],
                        wl[:, 64 * t : 64 * t + 64],
                        rcur[:, 512:1024],
                        start=(t == 0),
                        stop=(t == T - 1),
                    )

                dout = scp.tile([64, 1024], F32, name="dout", tag="dout")
                nc.vector.tensor_copy(dout[:, 0:512], d0[:])
                nc.vector.tensor_copy(dout[:, 512:1024], d1[:])
                nc.sync.dma_start(out_d.ap(), dout[:])

    nc.compile()
    return nc


_NC_CACHE = {}


def _get_nc():
    if "nc" not in _NC_CACHE:
        _NC_CACHE["nc"] = build_nc()
    return _NC_CACHE["nc"]


def kernel(
    img,
    w1, b1, w2, b2, w3, b3, w4, b4,
    g2, bb2, rm2, rv2, g3, bb3, rm3, rv3, g4, bb4, rm4, rv4,
    wl, bl,
):
    img = _np(img)
    w1, w2, w3, w4 = _np(w1), _np(w2), _np(w3), _np(w4)
    b1, b2, b3, b4 = _np(b1), _np(b2), _np(b3), _np(b4)
    wl, bl = _np(wl), _np(bl)

    s2, sh2 = _fold_bn(_np(g2), _np(bb2), _np(rm2), _np(rv2))
    s3, sh3 = _fold_bn(_np(g3), _np(bb3), _np(rm3), _np(rv3))
    s4, sh4 = _fold_bn(_np(g4), _np(bb4), _np(rm4), _np(rv4))
    for sh, s in ((sh2, s2), (sh3, s3), (sh4, s4)):
        if np.any(sh != 0):
            raise NotImplementedError("nonzero BN shift not supported")
        if np.any(s <= 0):
            raise NotImplementedError("nonpositive BN scale not supported")

    # fold BN scales into conv weights (scale > 0 commutes with lrelu) and biases
    w1t = _block_diag_taps(w1, L1["ni"])
    w2t = _block_diag_taps(w2, L2["ni"], col_scale=s2)
    w3t = _block_diag_taps(w3, L3["ni"], col_scale=s3)
    w4t = _block_diag_taps(w4, L4["ni"], col_scale=s4)
    biases = [
        _bias_vec(b1, L1["ni"]),
        _bias_vec(b2 * s2, L2["ni"]),
        _bias_vec(b3 * s3, L3["ni"]),
        _bias_vec(b4 * s4, L4["ni"]),
    ]
    biasp = np.concatenate([b.reshape(1, 128) for b in biases], axis=0)
    biasn = (-0.8 * biasp).astype(np.float32)

    # wl [1, T*128*64] -> [c=128, t, hw=64]
    wlt = np.ascontiguousarray(
        wl.reshape(T, 128, 64).transpose(1, 0, 2).reshape(128, T * 64)
    )

    nc = _get_nc()
    shared = {
        "w1t": w1t, "w2t": w2t, "w3t": w3t, "w4t": w4t,
        "biasp": biasp, "biasn": biasn, "wlt": wlt,
    }
    in_maps = [
        {**shared, "img": np.ascontiguousarray(img[16 * k : 16 * k + 16])}
        for k in range(N_CORES)
    ]
    res = run_bass_kernel_spmd(nc, in_maps, list(range(N_CORES)))
    _NC_CACHE["last_res"] = res

    sw = float(np.sum(wl, dtype=np.float64))
    logits = np.empty((B_FULL, 1), np.float32)
    for k in range(N_CORES):
        D = res.results[k]["D"].reshape(64, 16, 64)
        e = np.einsum("hbh->b", D).astype(np.float32)
        logits[16 * k : 16 * k + 16, 0] = (e + sw) * 0.5
    logits += bl.reshape(1, 1)
    return (1.0 / (1.0 + np.exp(-logits))).astype(np.float32)


if __name__ == "__main__":
    nc = build_nc()
    print("built ok:", len(nc.m.functions[0].instructions), "instructions")



# revision 13
# speedup vs baseline: 1.5600x; 1.5600x over previous
"""Trainium2 Bass kernel for nn_Discriminator_55800215109843.

Model: 4x (Conv2d k3 s2 p1 + LeakyReLU(0.2) [+ BatchNorm eval]) on
[128,3,128,128] -> [128,128,8,8], then a 50-step LIF neuron scan
(beta=0.95, thr=1, subtract reset) whose spike record feeds a linear
layer [409600 -> 1] + sigmoid.

Strategy (8 NeuronCores, pure data parallelism over batch, 16 imgs/core):

* Convs as tap-accumulation matmuls with dy-FOLDING: the padded input
  planes are replicated on spare partitions pre-shifted by one image
  row, so the dy taps ride in the contraction dim (K) and only the 3 dx
  taps remain as psum-accumulation passes.  All matmuls stay fp32
  (exact); BN (eval) folds into weights/biases on the host.
    L1: 3 bands (dy 0,1,2) x 24 rows -> K=72, 3 passes (was 9).
    L2/3/4: 2 bands -> K=128, 3 passes + 3 d2-passes (K=64) that reuse
    band0 rows+2 / band1 rows+1 (was 9 at quarter-width K).
  Even/odd activation tiles swap band order so each psum half writes
  partition-aligned, and every matmul uses tile_position (0,0).
* LeakyReLU(0.2) via lrelu(x) = x + 0.8*relu(-x): ACT Relu pass + one
  fused DVE scalar_tensor_tensor per psum half.
* LIF scan in negated fixed-point form: with P = 20*c, z = m - P the
  update is z' = beta*z - s; tracking negz = -z gives
      negz' = beta*negz + s        (one fused DVE STT, or gpsimd
                                    tensor_scalar + tensor_tensor(add)
                                    for a column slice)
      s     = (-negz > thr)        (one DVE STT with op1=is_gt)
  where thr = 1 - 20*c and negz_0 = 20*c are precomputed.  Spikes are
  exact {0,1} and feed the linear layer folded into the scan as 50
  accumulating fp32r PE matmuls against wl re-laid-out as [c, t, hw];
  the hw-diagonal of the [64,1024] psum recovers the dot product.
"""

import sys

sys.path.insert(0, "/opt/trn_rl_repo")

import numpy as np

import concourse.bass as bass
import concourse.mybir as mybir
import concourse.tile as tile
from concourse import bacc
from concourse.bass_utils import run_bass_kernel_spmd

F32 = mybir.dt.float32
F32R = mybir.dt.float32r
OP = mybir.AluOpType
AF = mybir.ActivationFunctionType

N_CORES = 8
B_FULL = 128
B_LOC = 16          # images per core
T = 50              # LIF steps
BETA = 0.95
S = 128             # input spatial
W1 = 129            # padded L1 plane (129x129: right/bottom pad cols unused)
W2, W3, W4 = 66, 34, 18
GP_COLS = 480       # scan columns handled by gpsimd (TS+TT add)


def _np(x):
    return np.ascontiguousarray(np.asarray(x, dtype=np.float32))


def _fold_bn(g, bb, rm, rv, eps=0.8):
    scale = g / np.sqrt(rv + eps)
    shift = bb - rm * scale
    return scale.astype(np.float32), shift.astype(np.float32)


def _l1_fold(w1):
    """w1 [16,3,3,3] -> dx-taps [3, 72, 128]: rows 24*dy + 3*i + c,
    cols 16*i + co, block-diag over 8 images."""
    t = np.zeros((3, 72, 128), np.float32)
    for dx in range(3):
        for d in range(3):
            blk = w1[:, :, d, dx].T.astype(np.float32)  # [3, 16]
            for i in range(8):
                t[dx, 24 * d + 3 * i : 24 * d + 3 * i + 3,
                  16 * i : 16 * i + 16] = blk
    return t


def _fold2(w, n_img, cs):
    """w [co,ci,3,3] -> (wae, wao [3,128,128], wb [3,64,128]).

    wae rows: 64*d + ci*i + c (d in {0,1});  wao: bands swapped
    (64*(1-d)); wb: d=2 only, rows ci*i + c.  cols: co*i + o, scaled by
    the folded BN scale cs[o]."""
    co, ci = w.shape[0], w.shape[1]
    wae = np.zeros((3, 128, 128), np.float32)
    wao = np.zeros((3, 128, 128), np.float32)
    wb = np.zeros((3, 64, 128), np.float32)
    for dx in range(3):
        for d in range(2):
            blk = w[:, :, d, dx].T.astype(np.float32) * cs[None, :]
            for i in range(n_img):
                r, c0 = ci * i, co * i
                wae[dx, 64 * d + r : 64 * d + r + ci, c0 : c0 + co] = blk
                wao[dx, 64 * (1 - d) + r : 64 * (1 - d) + r + ci, c0 : c0 + co] = blk
        blk2 = w[:, :, 2, dx].T.astype(np.float32) * cs[None, :]
        for i in range(n_img):
            wb[dx, ci * i : ci * i + ci, co * i : co * i + co] = blk2
    return wae, wao, wb


def _bias_vec(b, n_img):
    v = np.zeros(128, np.float32)
    co = b.shape[0]
    for i in range(n_img):
        v[i * co : (i + 1) * co] = b
    return v


def host_prep(inputs):
    """Build all device-side constant arrays from the raw model inputs."""
    w1 = _np(inputs["w1"])
    s2, sh2 = _fold_bn(_np(inputs["g2"]), _np(inputs["bb2"]), _np(inputs["rm2"]), _np(inputs["rv2"]))
    s3, sh3 = _fold_bn(_np(inputs["g3"]), _np(inputs["bb3"]), _np(inputs["rm3"]), _np(inputs["rv3"]))
    s4, sh4 = _fold_bn(_np(inputs["g4"]), _np(inputs["bb4"]), _np(inputs["rm4"]), _np(inputs["rv4"]))
    for sh, s in ((sh2, s2), (sh3, s3), (sh4, s4)):
        if np.any(sh != 0):
            raise NotImplementedError("nonzero BN shift not supported")
        if np.any(s <= 0):
            raise NotImplementedError("nonpositive BN scale not supported")

    w2ae, w2ao, w2b = _fold2(_np(inputs["w2"]), 4, s2)
    w3ae, w3ao, w3b = _fold2(_np(inputs["w3"]), 2, s3)
    w4ae, w4ao, w4b = _fold2(_np(inputs["w4"]), 1, s4)

    biases = [
        _bias_vec(_np(inputs["b1"]), 8),
        _bias_vec(_np(inputs["b2"]) * s2, 4),
        _bias_vec(_np(inputs["b3"]) * s3, 2),
        _bias_vec(_np(inputs["b4"]) * s4, 1),
    ]
    biasp = np.stack(biases, axis=0)                    # [4, 128]
    biasn = (-0.8 * biasp).astype(np.float32)

    wl = _np(inputs["wl"])
    wlt = np.ascontiguousarray(
        wl.reshape(T, 128, 64).transpose(1, 0, 2).reshape(128, T * 64)
    )
    return {
        "w1f": _l1_fold(w1),
        "w2ae": w2ae, "w2ao": w2ao, "w2b": w2b,
        "w3ae": w3ae, "w3ao": w3ao, "w3b": w3b,
        "w4ae": w4ae, "w4ao": w4ao, "w4b": w4b,
        "biasp": biasp, "biasn": biasn, "wlt": wlt,
    }


def build_nc(debug_dump=False):
    nc = bacc.Bacc("TRN2", target_bir_lowering=False, debug=False)

    # ---------------- DRAM I/O ----------------
    img_d = nc.dram_tensor("img", [B_LOC, 3, S, S], F32, kind="ExternalInput")
    w1f_d = nc.dram_tensor("w1f", [3, 72, 128], F32, kind="ExternalInput")
    wa_d, wb_d = {}, {}
    for li in (2, 3, 4):
        wa_d[li, 0] = nc.dram_tensor(f"w{li}ae", [3, 128, 128], F32, kind="ExternalInput")
        wa_d[li, 1] = nc.dram_tensor(f"w{li}ao", [3, 128, 128], F32, kind="ExternalInput")
        wb_d[li] = nc.dram_tensor(f"w{li}b", [3, 64, 128], F32, kind="ExternalInput")
    bp_d = nc.dram_tensor("biasp", [4, 128], F32, kind="ExternalInput")
    bn_d = nc.dram_tensor("biasn", [4, 128], F32, kind="ExternalInput")
    wl_d = nc.dram_tensor("wlt", [128, T * 64], F32R, kind="ExternalInput")
    out_d = nc.dram_tensor("D", [64, 1024], F32, kind="ExternalOutput")
    dbg_d = {}
    if debug_dump:
        dbg_d["x2_0"] = nc.dram_tensor("dbg_x2_0", [128, W2 * W2], F32, kind="ExternalOutput")
        dbg_d["x3_0"] = nc.dram_tensor("dbg_x3_0", [128, W3 * W3], F32, kind="ExternalOutput")
        dbg_d["x4_0"] = nc.dram_tensor("dbg_x4_0", [128, W4 * W4], F32, kind="ExternalOutput")
        dbg_d["ctile"] = nc.dram_tensor("dbg_ctile", [128, 1024], F32, kind="ExternalOutput")

    with tile.TileContext(nc) as tc:
        with (
            tc.tile_pool(name="const", bufs=1) as constp,
            tc.tile_pool(name="x2p", bufs=1) as x2p,
            tc.tile_pool(name="tmps", bufs=2) as tmps,
            tc.tile_pool(name="psum", bufs=3, space="PSUM") as psp,
            tc.tile_pool(name="psl4", bufs=1, space="PSUM") as psl4,
        ):
            # ---- constants needed during L1 ----
            w1sb = constp.tile([72, 3, 128], F32, name="w1sb", tag="w1sb")
            nc.sync.dma_start(w1sb[:], w1f_d.ap().transpose([1, 0, 2]))
            biasp = constp.tile([128, 4], F32, name="biasp", tag="biasp")
            nc.sync.dma_start(biasp[:], bp_d.ap().transpose([1, 0]))
            biasn = constp.tile([128, 4], F32, name="biasn", tag="biasn")
            nc.sync.dma_start(biasn[:], bn_d.ap().transpose([1, 0]))

            # ---- x2 dual-band tiles (band0/band1 swapped on odd tiles) ----
            x2d = [x2p.tile([128, W2 * W2], F32, name=f"x2_{i}", tag=f"x2_{i}")
                   for i in range(4)]
            for j, t in enumerate(x2d):
                v = t[:].rearrange("p (h w) -> p h w", w=W2)
                p0 = 0 if j % 2 == 0 else 64
                if debug_dump:
                    nc.vector.memset(t[:], 0.0)
                nc.vector.memset(v[p0 : p0 + 64, 0, :], 0.0)
                nc.vector.memset(v[p0 : p0 + 64, :, 0], 0.0)

            def band1_copy(t, W, R, C, odd):
                """Fill band1 = band0 shifted one plane row: dst rows 0..R-1,
                cols 0..C-1 <- src plane rows 1..R, cols 0..C-1.  Band0 is at
                partitions 0-63 (even tiles) / 64-127 (odd)."""
                v = t[:].rearrange("p (h w) -> p h w", w=W)
                if odd:
                    nc.sync.dma_start(v[0:64, 0:R, 0:C], v[64:128, 1 : R + 1, 0:C])
                else:
                    nc.sync.dma_start(v[64:128, 0:R, 0:C], v[0:64, 1 : R + 1, 0:C])

            def epilogue(ps, nfree, li, dsts):
                """lrelu(ps + bias): ACT relu pass + one STT per psum half.
                dsts = [(dst_ap, part_base), ...]"""
                rr = tmps.tile([128, 512], F32, name="relu_tmp", tag="relu_tmp")
                nc.scalar.activation(
                    rr[:, 0:nfree], ps[:, 0:nfree], AF.Relu,
                    bias=biasn[:, li : li + 1], scale=-0.8,
                )
                for dst, pb in dsts:
                    nc.vector.scalar_tensor_tensor(
                        dst, ps[pb : pb + 64, 0:nfree],
                        biasp[pb : pb + 64, li : li + 1],
                        rr[pb : pb + 64, 0:nfree],
                        OP.add, OP.add,
                    )

            # ---------------- L1 (scoped x1 planes) ----------------
            with tc.tile_pool(name="x1p", bufs=1) as x1p:
                x1t = [x1p.tile([72, W1 * W1], F32, name=f"x1_{g}", tag=f"x1_{g}")
                       for g in range(2)]
                for g in range(2):
                    v = x1t[g][:].rearrange("p (h w) -> p h w", w=W1)
                    base_off = g * 8 * 3 * S * S
                    # band d holds plane[y+d]; interior = img rows
                    src = bass.AP(tensor=img_d, offset=base_off,
                                  ap=[[S * S, 24], [S, 128], [1, 128]])
                    nc.sync.dma_start(v[0:24, 1:129, 1:129], src)
                    src = bass.AP(tensor=img_d, offset=base_off,
                                  ap=[[S * S, 24], [S, 128], [1, 128]])
                    nc.sync.dma_start(v[24:48, 0:128, 1:129], src)
                    src = bass.AP(tensor=img_d, offset=base_off + S,
                                  ap=[[S * S, 24], [S, 127], [1, 128]])
                    nc.sync.dma_start(v[48:72, 0:127, 1:129], src)
                    nc.vector.memset(v[0:24, 0, :], 0.0)   # top pad (band0)
                    nc.vector.memset(v[0:72, :, 0], 0.0)   # left pad cols

                for g in range(2):
                    v = x1t[g][:].rearrange("p (h w) -> p h w", w=W1)
                    for q in range(8):
                        ps = psp.tile([128, 512], F32, name="convps", tag="convps")
                        for dx in range(3):
                            rhs = v[0:72, 16 * q : 16 * q + 16 : 2, dx : dx + 127 : 2]
                            nc.tensor.matmul(
                                ps[:], w1sb[0:72, dx, :], rhs,
                                start=(dx == 0), stop=(dx == 2),
                                tile_position=(0, 0),
                            )
                        dsts = []
                        for h in range(2):
                            dv = x2d[2 * g + h][:].rearrange("p (h w) -> p h w", w=W2)
                            dsts.append(
                                (dv[64 * h : 64 * h + 64, 8 * q + 1 : 8 * q + 9, 1:65], 64 * h)
                            )
                        epilogue(ps, 512, 0, dsts)
                    # band1 fill for this group's two tiles (shift one row):
                    # copy only the initialized window plane rows 1..R,
                    # cols 0..C-1 (band1 is read at rows <= R-1, cols <= C-1)
                    for h in range(2):
                        band1_copy(x2d[2 * g + h], W2, 64, 65, h == 1)

            # ---------------- L2..L4 + scan ----------------
            with tc.tile_pool(name="rest", bufs=1) as rp:
                wa_sb, wb_sb = {}, {}
                for li in (2, 3, 4):
                    for par in (0, 1):
                        wt = rp.tile([128, 3, 128], F32, name=f"w{li}a{par}", tag=f"w{li}a{par}")
                        nc.sync.dma_start(wt[:], wa_d[li, par].ap().transpose([1, 0, 2]))
                        wa_sb[li, par] = wt
                    wt = rp.tile([64, 3, 128], F32, name=f"w{li}b", tag=f"w{li}b")
                    nc.sync.dma_start(wt[:], wb_d[li].ap().transpose([1, 0, 2]))
                    wb_sb[li] = wt
                wl = rp.tile([128, T * 64], F32R, name="wl", tag="wl")
                nc.sync.dma_start(wl[:], wl_d.ap())

                x3d = [rp.tile([128, W3 * W3], F32, name=f"x3_{i}", tag=f"x3_{i}")
                       for i in range(8)]
                x4d = [rp.tile([128, W4 * W4], F32, name=f"x4_{i}", tag=f"x4_{i}")
                       for i in range(16)]
                ctile = rp.tile([128, 1024], F32, name="ctile", tag="ctile")
                for j, t in enumerate(x3d):
                    v = t[:].rearrange("p (h w) -> p h w", w=W3)
                    p0 = 0 if j % 2 == 0 else 64
                    if debug_dump:
                        nc.vector.memset(t[:], 0.0)
                    nc.vector.memset(v[p0 : p0 + 64, 0, :], 0.0)
                    nc.vector.memset(v[p0 : p0 + 64, :, 0], 0.0)
                for j, t in enumerate(x4d):
                    v = t[:].rearrange("p (h w) -> p h w", w=W4)
                    p0 = 0 if j % 2 == 0 else 64
                    if debug_dump:
                        nc.vector.memset(t[:], 0.0)
                    nc.vector.memset(v[p0 : p0 + 64, 0, :], 0.0)
                    nc.vector.memset(v[p0 : p0 + 64, :, 0], 0.0)

                # ---- L2: 4 groups x 2 chunks of 512 ----
                for g2 in range(4):
                    v = x2d[g2][:].rearrange("p (h w) -> p h w", w=W2)
                    wa = wa_sb[2, g2 % 2]
                    off = 2 if g2 % 2 == 0 else 1
                    for q in range(2):
                        ps = psp.tile([128, 512], F32, name="convps", tag="convps")
                        for dx in range(3):
                            rhs = v[0:128, 32 * q : 32 * q + 32 : 2, dx : dx + 64 : 2]
                            nc.tensor.matmul(ps[:], wa[0:128, dx, :], rhs,
                                             start=(dx == 0), stop=False,
                                             tile_position=(0, 0))
                        for dx in range(3):
                            rhs = v[0:64, 32 * q + off : 32 * q + off + 32 : 2,
                                    dx : dx + 64 : 2]
                            nc.tensor.matmul(ps[:], wb_sb[2][0:64, dx, :], rhs,
                                             start=False, stop=(dx == 2),
                                             tile_position=(0, 0))
                        dsts = []
                        for h in range(2):
                            dv = x3d[2 * g2 + h][:].rearrange("p (h w) -> p h w", w=W3)
                            dsts.append(
                                (dv[64 * h : 64 * h + 64, 16 * q + 1 : 16 * q + 17, 1:33], 64 * h)
                            )
                        epilogue(ps, 512, 1, dsts)
                    for h in range(2):
                        band1_copy(x3d[2 * g2 + h], W3, 32, 33, h == 1)

                # ---- L3: 8 groups, one 256-chunk each ----
                for g3 in range(8):
                    v = x3d[g3][:].rearrange("p (h w) -> p h w", w=W3)
                    wa = wa_sb[3, g3 % 2]
                    off = 2 if g3 % 2 == 0 else 1
                    ps = psp.tile([128, 512], F32, name="convps", tag="convps")
                    for dx in range(3):
                        rhs = v[0:128, 0:32:2, dx : dx + 32 : 2]
                        nc.tensor.matmul(ps[:, 0:256], wa[0:128, dx, :], rhs,
                                         start=(dx == 0), stop=False,
                                         tile_position=(0, 0))
                    for dx in range(3):
                        rhs = v[0:64, off : off + 32 : 2, dx : dx + 32 : 2]
                        nc.tensor.matmul(ps[:, 0:256], wb_sb[3][0:64, dx, :], rhs,
                                         start=False, stop=(dx == 2),
                                         tile_position=(0, 0))
                    dsts = []
                    for h in range(2):
                        dv = x4d[2 * g3 + h][:].rearrange("p (h w) -> p h w", w=W4)
                        dsts.append((dv[64 * h : 64 * h + 64, 1:17, 1:17], 64 * h))
                    epilogue(ps, 256, 2, dsts)
                    for h in range(2):
                        band1_copy(x4d[2 * g3 + h], W4, 16, 17, h == 1)

                # ---- L4: 16 imgs, 64 cols each, 2 long-lived psum banks ----
                ps4 = [psl4.tile([128, 512], F32, name=f"ps4_{i}", tag=f"ps4_{i}")
                       for i in range(2)]
                for ii in range(16):
                    v = x4d[ii][:].rearrange("p (h w) -> p h w", w=W4)
                    wa = wa_sb[4, ii % 2]
                    off = 2 if ii % 2 == 0 else 1
                    out = ps4[ii // 8][:, 64 * (ii % 8) : 64 * (ii % 8) + 64]
                    for dx in range(3):
                        rhs = v[0:128, 0:16:2, dx : dx + 16 : 2]
                        nc.tensor.matmul(out, wa[0:128, dx, :], rhs,
                                         start=(dx == 0), stop=False,
                                         tile_position=(0, 0), skip_group_check=True)
                    for dx in range(3):
                        rhs = v[0:64, off : off + 16 : 2, dx : dx + 16 : 2]
                        nc.tensor.matmul(out, wb_sb[4][0:64, dx, :], rhs,
                                         start=False, stop=(dx == 2),
                                         tile_position=(0, 0), skip_group_check=True)
                for pb in range(2):
                    epilogue(ps4[pb], 512, 3,
                             [(ctile[0:64, 512 * pb : 512 * pb + 512], 0),
                              (ctile[64:128, 512 * pb : 512 * pb + 512], 64)])

                if debug_dump:
                    nc.sync.dma_start(dbg_d["x2_0"].ap(), x2d[0][:])
                    nc.sync.dma_start(dbg_d["x3_0"].ap(), x3d[0][:])
                    nc.sync.dma_start(dbg_d["x4_0"].ap(), x4d[0][:])
                    nc.sync.dma_start(dbg_d["ctile"].ap(), ctile[:])

                # ---------------- LIF scan (negz form) + folded linear ----
                with (
                    tc.tile_pool(name="scan", bufs=1) as scp,
                    tc.tile_pool(name="psd", bufs=1, space="PSUM") as psd,
                ):
                    za = scp.tile([128, 1024], F32, name="za", tag="za")
                    zb = scp.tile([128, 1024], F32, name="zb", tag="zb")
                    thr = scp.tile([128, 1024], F32, name="thr", tag="thr")
                    gt = scp.tile([128, 1024], F32, name="gt", tag="gt")
                    sg = [scp.tile([128, 1024], F32R, name=f"s{i}", tag=f"s{i}")
                          for i in range(2)]
                    d0 = psd.tile([64, 512], F32, name="d0", tag="d0")
                    d1 = psd.tile([64, 512], F32, name="d1", tag="d1")

                    X = 1024 - GP_COLS
                    nc.vector.tensor_scalar(za[:], ctile[:], 20.0, None, OP.mult)
                    nc.vector.tensor_scalar(thr[:], ctile[:], -20.0, 1.0, OP.mult, OP.add)
                    nc.vector.memset(sg[0][:].bitcast(F32), 0.0)

                    zc, zn = za, zb
                    for k in range(1, T + 1):
                        spf = sg[(k - 1) % 2][:].bitcast(F32)
                        sc = sg[k % 2]
                        # negz' = beta*negz + s_prev
                        if GP_COLS:
                            nc.gpsimd.tensor_scalar(
                                gt[:, X:1024], zc[:, X:1024], BETA, None, OP.mult)
                            nc.gpsimd.tensor_tensor(
                                zn[:, X:1024], gt[:, X:1024], spf[:, X:1024], OP.add)
                        nc.vector.scalar_tensor_tensor(
                            zn[:, 0:X], zc[:, 0:X], BETA, spf[:, 0:X],
                            OP.mult, OP.add)
                        # s = (-negz' > thr); written as f32r for the matmul
                        nc.vector.scalar_tensor_tensor(
                            sc[:, 0:X], zn[:, 0:X], -1.0, thr[:, 0:X],
                            OP.mult, OP.is_gt)
                        nc.vector.scalar_tensor_tensor(
                            sc[:, X:1024], zn[:, X:1024], -1.0, thr[:, X:1024],
                            OP.mult, OP.is_gt)
                        # D += wl_t^T @ s_t
                        nc.tensor.matmul(d0[:], wl[:, 64 * (k - 1) : 64 * k],
                                         sc[:, 0:512],
                                         start=(k == 1), stop=(k == T))
                        nc.tensor.matmul(d1[:], wl[:, 64 * (k - 1) : 64 * k],
                                         sc[:, 512:1024],
                                         start=(k == 1), stop=(k == T))
                        zc, zn = zn, zc

                    dout = scp.tile([64, 1024], F32, name="dout", tag="dout")
                    nc.vector.tensor_copy(dout[:, 0:512], d0[:])
                    nc.vector.tensor_copy(dout[:, 512:1024], d1[:])
                    nc.sync.dma_start(out_d.ap(), dout[:])

    nc.compile()
    return nc


_NC_CACHE = {}


def _get_nc():
    if "nc" not in _NC_CACHE:
        _NC_CACHE["nc"] = build_nc()
    return _NC_CACHE["nc"]


def kernel(
    img,
    w1, b1, w2, b2, w3, b3, w4, b4,
    g2, bb2, rm2, rv2, g3, bb3, rm3, rv3, g4, bb4, rm4, rv4,
    wl, bl,
):
    inputs = {
        "w1": w1, "b1": b1, "w2": w2, "b2": b2, "w3": w3, "b3": b3,
        "w4": w4, "b4": b4,
        "g2": g2, "bb2": bb2, "rm2": rm2, "rv2": rv2,
        "g3": g3, "bb3": bb3, "rm3": rm3, "rv3": rv3,
        "g4": g4, "bb4": bb4, "rm4": rm4, "rv4": rv4,
        "wl": wl,
    }
    shared = host_prep(inputs)
    img = _np(img)
    bl = _np(bl)

    nc = _get_nc()
    in_maps = [
        {**shared, "img": np.ascontiguousarray(img[16 * k : 16 * k + 16])}
        for k in range(N_CORES)
    ]
    res = run_bass_kernel_spmd(nc, in_maps, list(range(N_CORES)))
    _NC_CACHE["last_res"] = res

    logits = np.empty((B_FULL, 1), np.float32)
    for k in range(N_CORES):
        D = res.results[k]["D"].reshape(64, 16, 64)
        e = np.einsum("hbh->b", D).astype(np.float32)
        logits[16 * k : 16 * k + 16, 0] = e
    logits += bl.reshape(1, 1)
    return (1.0 / (1.0 + np.exp(-logits))).astype(np.float32)


if __name__ == "__main__":
    nc = build_nc()
    print("built ok")
